# revision 36
# baseline (speedup 1.0000x reference)
"""Ising-model energy kernel for 8 Trainium2 NeuronCores.

result = 0.25*S0 - 0.5*(Qup + Qdiag + S2)
  S0    = sum(A)                          (A = info_mtx)
  Qup   = sum over off-diagonal 128x128 tiles (t > g) of s_g^T A_tile s_t
  Qdiag = strict-upper part of the 64 diagonal tiles (host, fp64)
  S2    = sum_i A[i,i] s_i                (host, fp64)

Sharding: row-shard A into 8 slabs [1024, 8192], one per core.  The slab is
cast to fp8 e4m3 on host (tolerance is 2e-2; fp8 rounding error on the big
sums is O(100) against an answer of ~8.4e6) and streamed on a single HWDGE
queue in consumption order — the two HWDGE queues share one ~400GB/s HBM
pipe and starve each other unpredictably, so one FIFO queue is both faster
to start and deterministic.  Pairs of 128-row blocks ride side by side; the
last pair is split 1.5MB/0.5MB so the tail work after the final DMA
completion is minimal.

Each pair is the *moving* operand of DoubleRow fp8 matmuls (contraction 256
= 2 blocks x 128 rows, 2 elements per PE cell) against a tiny stationary
holding [s_block0 | s_block1 | ones] column triplets, so the PE consumes
two A elements per lane per cycle.  Each of the 16 column-groups of 512
columns accumulates into one of 8 PSUM banks (two groups per bank, packed
into disjoint 16-row halves of the 32-row output; the stationary's zero
columns make the off-half rows accumulate +0).

Device output per core is [32, 4096] fp16 (fp8 out was tried: the e4m3
grid step of 16 at colsum magnitude ~128 biases the sums, rel err 7e-3):
within the 16-row half of column
group 8q+k, rows 3p / 3p+1 are the matvec u of blocks 2p / 2p+1 and row
3p+2 is the pair's column sum.  Host does the O(N)-sized mask/reduce and
the exact diag-tile terms.
"""

import numpy as np

N = 8192
NCORES = 8
ROWS = N // NCORES   # 1024 rows per core
BLK = 128            # partition block
NB = ROWS // BLK     # 8 row blocks per core
NPAIR = NB // 2      # 4 DoubleRow pairs per core
NT = N // BLK        # 64 column tiles (mask granularity)
GW = 512             # column-group width (one PSUM bank of fp32)
NG = N // GW         # 16 column groups
NBANK = 8            # PSUM banks used; 2 groups per bank
SPLIT = 12           # last pair: groups 0-11 in segment a, 12-15 in b

_NC_CACHE = None
LAST_EXEC_NS = None
LAST_RESULTS = None


def _build_nc():
    import concourse.bass as bass
    import concourse.tile as tile
    from concourse.tile_rust import add_dep_helper
    from concourse import mybir

    f32 = mybir.dt.float32
    f16 = mybir.dt.float16
    f8 = mybir.dt.float8e4
    dr = mybir.MatmulPerfMode.DoubleRow
    nc = bass.Bass()
    WTW = NPAIR * 2 * 64
    # pair 0 carries the stationary tiles in its tail: both are needed at
    # the same time and merging them frees one HWDGE lane for a third
    # output DMA
    a0 = nc.dram_tensor("a0", [BLK, 2 * N + WTW], f8, kind="ExternalInput")
    a = nc.dram_tensor("a", [NPAIR - 2, BLK, 2 * N], f8, kind="ExternalInput")
    a3a = nc.dram_tensor("a3a", [BLK, 2 * SPLIT * GW], f8, kind="ExternalInput")
    a3b = nc.dram_tensor("a3b", [BLK, 2 * (NG - SPLIT) * GW], f8, kind="ExternalInput")
    o = nc.dram_tensor("o", [32, NBANK * GW], f16, kind="ExternalOutput")

    with tile.TileContext(nc) as tc:
        with (
            tc.tile_pool(name="data", bufs=1) as data,
            tc.tile_pool(name="psum", bufs=1, space="PSUM") as psum_pool,
        ):
            # Single queue, issue order == consumption order.
            sl0 = data.tile([BLK, 2 * N + WTW], f8, tag="slab0")
            loads = [nc.sync.dma_start(out=sl0, in_=a0[:, :])]
            wt = sl0[:, 2 * N :]
            slabs = [sl0[:, : 2 * N].rearrange("r (h n) -> r h n", h=2)]
            for p in range(1, NPAIR - 1):
                sl = data.tile([BLK, 2 * N], f8, tag=f"slab{p}", name=f"slab{p}")
                loads.append(nc.sync.dma_start(out=sl, in_=a[p - 1, :, :]))
                slabs.append(sl.rearrange("r (h n) -> r h n", h=2))
            s3a = data.tile([BLK, 2 * SPLIT * GW], f8, tag="s3a")
            loads.append(nc.sync.dma_start(out=s3a, in_=a3a[:, :]))
            s3b = data.tile([BLK, 2 * (NG - SPLIT) * GW], f8, tag="s3b")
            loads.append(nc.sync.dma_start(out=s3b, in_=a3b[:, :]))
            seg_a = s3a.rearrange("r (h n) -> r h n", h=2)
            seg_b = s3b.rearrange("r (h n) -> r h n", h=2)
            w3 = wt.rearrange("r (s h m) -> r s h m", s=NPAIR * 2, h=2)

            pbank = [
                psum_pool.tile([32, GW], f32, tag=f"pb{k}", name=f"pb{k}")
                for k in range(NBANK)
            ]

            def rhs_for(p, g):
                if p < NPAIR - 1:
                    return slabs[p][:, :, GW * g : GW * (g + 1)]
                if g < SPLIT:
                    return seg_a[:, :, GW * g : GW * (g + 1)]
                return seg_b[:, :, GW * (g - SPLIT) : GW * (g - SPLIT + 1)]

            for p in range(NPAIR):
                # Last pair: banks 0-3 finish from segment a (their q=1
                # groups 8-11 live there), banks 4-7 from segment b, so the
                # per-bank stops retire as data lands and the copies overlap
                # the remaining matmuls.
                if p < NPAIR - 1:
                    qk = [(q, k) for q in range(2) for k in range(NBANK)]
                else:
                    # bank-major so each bank's stop retires as early as
                    # possible and the cast chain starts sooner
                    qk = [(q, k) for k in range(NBANK) for q in range(2)]
                for q, k in qk:
                    g = NBANK * q + k
                    last_mm = nc.tensor.matmul(
                        pbank[k][:, :],
                        w3[:, 2 * p + q, :, :],
                        rhs_for(p, g),
                        start=(p == 0 and q == 0),
                        stop=(p == NPAIR - 1 and q == 1),
                        perf_mode=dr,
                    )

            out_sb = data.tile([32, NBANK * GW], f16, tag="out")
            cps = []
            for k in range(NBANK):
                cps.append(
                    nc.vector.tensor_copy(
                        out_sb[:, GW * k : GW * (k + 1)], pbank[k][:, :]
                    )
                )

            # The 32-partition out_sb tile only drives 4 of 16 SBUF ports, so
            # the output transfer runs at ~1/4 line rate (~2.5us total).
            # Split it 4/3/1: the post-cast7 tail is bounded by the LAST
            # DMA's size (measured: 3/3/2 with a 2-bank straggler was 0.6us
            # worse), so the straggler is a single 32KB bank (5 input + 3
            # output HW DMAs = 8 lanes, no lane-reuse waits).
            c1, c2 = 4 * GW, 7 * GW
            od1 = nc.sync.dma_start(out=o[:, :c1], in_=out_sb[:, :c1])
            od1b = nc.sync.dma_start(out=o[:, c1:c2], in_=out_sb[:, c1:c2])
            od2 = nc.sync.dma_start(out=o[:, c2:], in_=out_sb[:, c2:])
            # The kernel-tail drain may carry only one sync wait; give SP a
            # 1-wait nop per otherwise-unobserved final semaphore tick so the
            # drain ends up with at most one wait left.
            for dep in loads + [last_mm, cps[-1], od1, od1b, od2]:
                nop = nc.sync.nop()
                add_dep_helper(nop.ins, dep.ins, sync=True, reason="tail sem absorb")
    return nc


def _pack_inputs(A: np.ndarray, s: np.ndarray):
    import ml_dtypes

    f8 = ml_dtypes.float8_e4m3
    s_blocks = s.reshape(NT, BLK)  # s_blocks[g, i] = s[128*g + i]
    in_maps = []
    for d in range(NCORES):
        a8 = A[d * ROWS : (d + 1) * ROWS].astype(f8)
        # pair p holds blocks 2p (h=0) and 2p+1 (h=1) side by side
        a8 = a8.reshape(NPAIR, 2, BLK, N).transpose(0, 2, 1, 3).reshape(
            NPAIR, BLK, 2 * N
        )
        # last pair split into contiguous column segments [0,SPLIT*GW) and
        # [SPLIT*GW, N), h-major within each segment
        p3 = a8[-1].reshape(BLK, 2, NG, GW)
        a3a = np.ascontiguousarray(
            p3[:, :, :SPLIT].reshape(BLK, 2 * SPLIT * GW)
        )
        a3b = np.ascontiguousarray(
            p3[:, :, SPLIT:].reshape(BLK, 2 * (NG - SPLIT) * GW)
        )
        W = np.zeros((BLK, NPAIR * 2 * 64), dtype=f8)
        for p in range(NPAIR):
            s0 = s_blocks[d * NB + 2 * p].astype(f8)
            s1 = s_blocks[d * NB + 2 * p + 1].astype(f8)
            for q in range(2):
                base = 64 * (2 * p + q) + 16 * q + 3 * p
                W[:, base + 0] = s0        # h=0 slot of out row 16q+3p
                W[:, base + 32 + 1] = s1   # h=1 slot of out row 16q+3p+1
                W[:, base + 2] = 1.0       # colsum row gets both halves
                W[:, base + 32 + 2] = 1.0
        in_maps.append(
            {
                "a0": np.ascontiguousarray(
                    np.concatenate([a8[0], W], axis=1)
                ),
                "a": np.ascontiguousarray(a8[1:-1]),
                "a3a": a3a,
                "a3b": a3b,
            }
        )
    return in_maps


def kernel(info_mtx: np.ndarray, state: np.ndarray, _trace: bool = False) -> np.ndarray:
    global _NC_CACHE, LAST_EXEC_NS, LAST_RESULTS

    A = np.ascontiguousarray(np.asarray(info_mtx, dtype=np.float32))
    s = np.ascontiguousarray(np.asarray(state, dtype=np.float32))

    in_maps = _pack_inputs(A, s)

    if _NC_CACHE is None:
        _NC_CACHE = _build_nc()
    from concourse.bass_utils import run_bass_kernel_spmd

    res = run_bass_kernel_spmd(_NC_CACHE, in_maps, list(range(NCORES)), trace=_trace)
    LAST_EXEC_NS = res.exec_time_ns
    LAST_RESULTS = res

    s64 = s.astype(np.float64)
    # Decode: o[16q + 3p + r, 512k + off] covers column j = 512*(8q+k) + off;
    # r=0 -> u of block 2p, r=1 -> u of block 2p+1, r=2 -> pair column sum.
    U = np.empty((NCORES * NB, N), np.float64)
    S0 = 0.0
    urow_idx = [r for p in range(NPAIR) for r in (3 * p, 3 * p + 1)]
    for d in range(NCORES):
        oq = res.results[d]["o"].astype(np.float64).reshape(2, 16, NBANK, GW)
        U[d * NB : (d + 1) * NB] = (
            oq[:, urow_idx].transpose(1, 0, 2, 3).reshape(NB, N)
        )
        S0 += oq[:, 2::3].sum()

    # Mask at 128-column-tile granularity: block g contributes tiles t > g.
    per_tile = (U * s64[None, :]).reshape(NT, NT, BLK).sum(axis=2)
    Qup = np.triu(per_tile, k=1).sum()

    Qdiag = 0.0
    for g in range(NT):
        blk = A[g * BLK : (g + 1) * BLK, g * BLK : (g + 1) * BLK].astype(np.float64)
        sb = s64[g * BLK : (g + 1) * BLK]
        Qdiag += sb @ (np.triu(blk, 1) @ sb)
    S2 = float(np.diagonal(A).astype(np.float64) @ s64)

    result = 0.25 * S0 - 0.5 * (Qup + Qdiag + S2)
    return np.asarray(result, dtype=np.float32)
